# revision 17
# baseline (speedup 1.0000x reference)
"""Trainium2 Bass kernel for MultiHeadSelfAttention with RoPE.

Problem: x[2, 2048, 1024] @ W_qkv[1024, 3072] -> rope(q,k) -> softmax(q k^T/8) v
         -> out @ W_out[1024, 1024].

Sharding (8 cores): batch (2-way) x head-group (4-way, 4 heads each).
Each core computes a partial output [2048, 1024] = attnout_heads @ W_out_rows;
host sums the 4 head-group partials per batch.

v2 design (vs v1):
  - scores: two concurrent K=64 matmuls via PE row-tiling (head A on array
    rows 0:63, head B on rows 64:127, auto tile_position from base
    partitions) instead of zero-padded K=128 -> ~2x PE scores time.
  - attn@v: fp8e4 DoubleRow matmuls (2 sk-tiles = K=256 virtual per MM at
    0.5 cyc/row) with the softmax-denominator ones-column aug (M=65).
    exp() writes fp8e4 directly from PSUM scores.
  - exp: one ACT instr per (sk, both heads) = [128, 1024], ping-pong on a
    4-bank PSUM scores tile so scores(sk+2) overlap exp(sk).
  - bf16 everywhere else (q/k/att_o/W_out); f32 PSUM accumulation.
  - in-order engine queues: emission order interleaves v-proj into
    attention(g0,ch0), pair-1 projection into attention(g0,ch1..3), and the
    output projection into attention(g1), so everything hides under the
    ScalarE exp stream (the bottleneck at ~1.15us per [128,1024] tile).
"""

import sys

if "/opt/trn_rl_repo" not in sys.path:
    sys.path.insert(0, "/opt/trn_rl_repo")

import numpy as np

B, S, E = 2, 2048, 1024
ATT = 1024
H = 16
D = 64
HG = 4            # head groups (cores per batch)
HPG = H // HG     # heads per core = 4
PAIRS = HPG // 2  # head pairs per core = 2
ROPE_THETA = 10000.0
N_CORES = 8

SQ = 512          # sq chunk
N_CH = S // SQ    # 4 chunks
N_SK = S // 128   # 16 sk tiles
N_SP = N_SK // 2  # 8 sk-tile pairs
EK = E // 128     # 8 contraction tiles over embedding dim
# -ln(64): numerator and denominator share the shift so softmax is unchanged.
# Keeps fp8e4 exp below 448 up to score 82 (observed max 62.3); drops only
# ~0.1% of softmax mass into the subnormal/zero range.
EXP_BIAS = -4.1588830833596715

_BUILT = {}
DBG = False


def _build_program():
    import concourse.bacc as bacc
    import concourse.tile as tile
    import concourse.mybir as mybir

    f32 = mybir.dt.float32
    bf16 = mybir.dt.bfloat16
    fp8 = mybir.dt.float8e4
    AF = mybir.ActivationFunctionType
    DR = mybir.MatmulPerfMode.DoubleRow
    ALU = mybir.AluOpType

    nc = bacc.Bacc(
        "TRN2",
        target_bir_lowering=False,
        debug=False,
        enable_asserts=False,
        num_devices=N_CORES,
    )

    xT = nc.dram_tensor("xT", [E, S], bf16, kind="ExternalInput").ap()
    w_qk = nc.dram_tensor("w_qk", [E, 2 * HPG * D], bf16, kind="ExternalInput").ap()
    w_v = nc.dram_tensor("w_v", [E, HPG * D], bf16, kind="ExternalInput").ap()
    w_o = nc.dram_tensor("w_o", [HPG * D, E], bf16, kind="ExternalInput").ap()
    cos_t = nc.dram_tensor("cos_t", [128, S], f32, kind="ExternalInput").ap()
    sin_t = nc.dram_tensor("sin_t", [128, S], f32, kind="ExternalInput").ap()
    mswap = nc.dram_tensor("mswap", [128, 128], bf16, kind="ExternalInput").ap()
    out = nc.dram_tensor("out", [S, E], bf16, kind="ExternalOutput").ap()
    if DBG:
        d_qT = nc.dram_tensor("d_qT", [128, S], bf16, kind="ExternalOutput").ap()
        d_kT = nc.dram_tensor("d_kT", [128, S], bf16, kind="ExternalOutput").ap()
        d_v = nc.dram_tensor("d_v", [128, N_SK * HPG * 65], bf16, kind="ExternalOutput").ap()
        d_e = nc.dram_tensor("d_e", [128, 2 * SQ], bf16, kind="ExternalOutput").ap()
        d_oT = nc.dram_tensor("d_oT", [65, 2 * SQ], f32, kind="ExternalOutput").ap()
        d_ao = nc.dram_tensor("d_ao", [128, 2 * S], bf16, kind="ExternalOutput").ap()

    with tile.TileContext(nc) as tc:
        with (
            tc.tile_pool(name="const", bufs=1) as constp,
            tc.tile_pool(name="xt", bufs=1) as xtp,
            tc.tile_pool(name="wqk", bufs=1) as wqkp,
            tc.tile_pool(name="trig", bufs=1) as trigp,
            tc.tile_pool(name="qkT", bufs=1) as qkTp,
            tc.tile_pool(name="vdr", bufs=1) as vp,
            tc.tile_pool(name="attnout", bufs=1) as aop,
            tc.tile_pool(name="wo", bufs=1) as wop,
            tc.tile_pool(name="ropes", bufs=2) as ropep,
            tc.tile_pool(name="exps", bufs=1) as expp,
            tc.tile_pool(name="norm", bufs=2) as rcp,
            tc.tile_pool(name="osb", bufs=3) as osbp,
            tc.tile_pool(name="ps", bufs=1, space="PSUM") as psp,
        ):
            # ---------------- static tiles ----------------
            msw_sb = constp.tile([128, 128], bf16, tag="msw")
            onesrow = constp.tile([65, 64], bf16, tag="onesrow")
            bias_t = constp.tile([128, 1], f32, tag="bias")
            nc.gpsimd.memset(onesrow[64:65, :], 1.0)
            nc.gpsimd.memset(bias_t[:], EXP_BIAS)

            # q'/k' per pair: [128, S] bf16; rows 0:64 head A dims, 64:128 head B
            qT = [qkTp.tile([128, S], bf16, tag=f"qT{g}", name=f"qT{g}") for g in range(PAIRS)]
            kT = [qkTp.tile([128, S], bf16, tag=f"kT{g}", name=f"kT{g}") for g in range(PAIRS)]
            # v in bf16: [ki, st, head, 65]; cols 0:64 = v dims, col 64 = ones
            # (softmax-denominator aug: oT row 64 accumulates sum of weights)
            v_c = vp.tile([128, N_SK, HPG, 65], bf16, tag="vc")
            nc.gpsimd.memset(v_c[:, :, :, 64:65], 1.0)
            # normalized attention output per pair [128 (pair dims), S] bf16
            att_o = [aop.tile([128, S], bf16, tag=f"ao{g}", name=f"ao{g}") for g in range(PAIRS)]
            wo_sb = [wop.tile([128, E], bf16, tag=f"wo{g}", name=f"wo{g}") for g in range(PAIRS)]

            cos_sb = trigp.tile([128, S], f32, tag="cos")
            sin_sb = trigp.tile([128, S], f32, tag="sin")
            wqk_all = wqkp.tile([128, EK, 2 * HPG * D], bf16, tag="wqk")
            wv_all = wqkp.tile([128, EK, HPG * D], bf16, tag="wv")
            xt_all = xtp.tile([128, EK, S], bf16, tag="xt")

            # ---------------- DMA (consumption order) ----------------
            nc.sync.dma_start(msw_sb[:], mswap[:])
            wqk_d = w_qk.rearrange("(ek p) c -> p ek c", p=128)
            xt_d = xT.rearrange("(ek p) s -> p ek s", p=128)
            nc.sync.dma_start(wqk_all[:], wqk_d)
            nc.sync.dma_start(xt_all[:, :, 0:512], xt_d[:, :, 0:512])
            nc.sync.dma_start(cos_sb[:, 0:512], cos_t[:, 0:512])
            nc.sync.dma_start(sin_sb[:, 0:512], sin_t[:, 0:512])
            for c in range(1, 4):
                csl = slice(512 * c, 512 * (c + 1))
                nc.sync.dma_start(xt_all[:, :, csl], xt_d[:, :, csl])
                nc.sync.dma_start(cos_sb[:, csl], cos_t[:, csl])
                nc.sync.dma_start(sin_sb[:, csl], sin_t[:, csl])
            nc.sync.dma_start(wv_all[:], w_v.rearrange("(ek p) c -> p ek c", p=128))
            for g in range(PAIRS):
                nc.sync.dma_start(wo_sb[g][:], w_o[128 * g : 128 * (g + 1), :])

            wqk_sb = [wqk_all[:, e, :] for e in range(EK)]
            xt_sb = [xt_all[:, e, :] for e in range(EK)]
            wv_sb = [wv_all[:, e, :] for e in range(EK)]

            # ---------------- helpers ----------------
            rope_pend = []

            def emit_proj_group(g, ti, c, evac_eng):
                """One 512-col chunk of the q/k projection for pair g."""
                sl = slice(512 * c, 512 * (c + 1))
                coff = ti * HPG * D + 128 * g
                pp = psp.tile([128, 512], f32, tag="pj", bufs=2, name="pp")
                for e in range(EK):
                    nc.tensor.matmul(
                        pp[:],
                        wqk_sb[e][:, coff : coff + 128],
                        xt_sb[e][:, sl],
                        start=(e == 0),
                        stop=(e == EK - 1),
                    )
                raw = ropep.tile([128, 512], bf16, tag="raw", name="raw")
                if evac_eng == "scalar":
                    nc.scalar.copy(raw[:], pp[:])
                else:
                    nc.vector.tensor_copy(raw[:], pp[:])
                dest = (qT, kT)[ti][g]
                rope_pend.append((dest, sl, raw))
                if len(rope_pend) > 1:
                    rope_tail()

            def rope_tail():
                (dest, sl, raw) = rope_pend.pop(0)
                rp = psp.tile([128, 512], f32, tag="pj", bufs=2, name="rp")
                nc.tensor.matmul(rp[:], msw_sb[:], raw[:], start=True, stop=True)
                t2 = ropep.tile([128, 512], f32, tag="t2")
                nc.vector.tensor_mul(t2[:], raw[:], cos_sb[:, sl])
                t1 = ropep.tile([128, 512], f32, tag="t1")
                nc.vector.tensor_mul(t1[:], rp[:], sin_sb[:, sl])
                nc.gpsimd.tensor_tensor(dest[:, sl], t1[:], t2[:], ALU.add)

            def emit_v_group(st):
                """v projection for sk-tile st (all 4 heads), into v_c bf16."""
                vp_ps = psp.tile([128, HPG * D], f32, tag="pj", bufs=2, name="vps")
                for e in range(EK):
                    nc.tensor.matmul(
                        vp_ps[:],
                        xt_sb[e][:, 128 * st : 128 * (st + 1)],
                        wv_sb[e][:],
                        start=(e == 0),
                        stop=(e == EK - 1),
                    )
                nc.vector.tensor_copy(
                    v_c[:, st, :, 0:64],
                    vp_ps.rearrange("p (h c) -> p h c", h=HPG),
                )

            def emit_outproj_tile(st, n, eng):
                """out[st*128:(st+1)*128, n*512:(n+1)*512] = attn_out @ W_out."""
                ssl = slice(128 * st, 128 * (st + 1))
                nsl = slice(512 * n, 512 * (n + 1))
                op = psp.tile([128, 512], f32, tag="pj", bufs=2, name="op")
                for g in range(PAIRS):
                    nc.tensor.matmul(
                        op[:],
                        att_o[g][:, ssl],
                        wo_sb[g][:, nsl],
                        start=(g == 0),
                        stop=(g == PAIRS - 1),
                    )
                ot = osbp.tile([128, 512], bf16, tag="ot")
                nc.vector.tensor_copy(ot[:], op[:])
                nc.sync.dma_start(out[ssl, nsl], ot[:])

            def attention_chunk(g, ch, fillers):
                csl = slice(SQ * ch, SQ * (ch + 1))
                sps = psp.tile([128, 2, 2, SQ], f32, tag="sps", name="sps")
                oT = [
                    psp.tile([65, SQ], f32, tag=f"oT{h}", name=f"oT{h}")
                    for h in range(2)
                ]
                e_cur = None
                fi = 0
                for sk in range(N_SK):
                    par = sk % 2
                    sksl = slice(128 * sk, 128 * (sk + 1))
                    for h in range(2):
                        pb = 64 * h
                        nc.tensor.matmul(
                            sps[:, par, h, :],
                            kT[g][pb : pb + 64, sksl],
                            qT[g][pb : pb + 64, csl],
                            start=True,
                            stop=True,
                        )
                    e_cur = expp.tile([128, 2, SQ], bf16, tag="e", bufs=2, name="ecur")
                    nc.scalar.activation(
                        e_cur[:],
                        sps[:, par, :, :],
                        AF.Exp,
                        scale=0.125,
                        bias=bias_t[:],
                    )
                    if fi < len(fillers) and sk % (N_SK // len(fillers)) == (N_SK // len(fillers)) - 1:
                        fillers[fi]()
                        fi += 1
                    if DBG and g == 0 and ch == 0 and sk == 0:
                        nc.sync.dma_start(d_e[:], e_cur.rearrange("p a b -> p (a b)"))
                    for h in range(2):
                        hh = 2 * g + h
                        nc.tensor.matmul(
                            oT[h][:],
                            v_c[:, sk, hh, 0:65],
                            e_cur[:, h, :],
                            start=(sk == 0),
                            stop=(sk == N_SK - 1),
                        )
                while fi < len(fillers):
                    fillers[fi]()
                    fi += 1
                # ---- normalize: row 64 of oT holds the denominators ----
                if DBG and g == 0 and ch == 0:
                    for h in range(2):
                        dt_ = rcp.tile([65, SQ], f32, tag="dbg", name="dbgt")
                        nc.vector.tensor_copy(dt_[:], oT[h][:])
                        nc.sync.dma_start(d_oT[:, SQ * h : SQ * (h + 1)], dt_[:])
                for h in range(2):
                    o_s = rcp.tile([65, SQ], bf16, tag=f"o{h}", name=f"os{h}")
                    nc.vector.tensor_copy(o_s[:], oT[h][:])
                    db = psp.tile([64, SQ], f32, tag=f"oT{h}", name=f"db{h}")
                    nc.tensor.matmul(
                        db[:], onesrow[64:65, :], o_s[64:65, :], start=True, stop=True
                    )
                    rb = rcp.tile([64, SQ], f32, tag=f"rb{h}", name=f"rb{h}")
                    nc.vector.reciprocal_approx_fast(rb[:], db[:])
                    if h == 0:
                        nc.vector.tensor_mul(
                            att_o[g][0:64, csl], o_s[0:64, :], rb[:]
                        )
                    else:
                        aoB = rcp.tile([64, SQ], bf16, tag="aoB", name="aoB")
                        nc.vector.tensor_mul(aoB[:], o_s[0:64, :], rb[:])
                        nc.sync.dma_start(att_o[g][64:128, csl], aoB[:])

            # ---------------- phase 1: pair-0 projection + rope ----------------
            for ti in range(2):          # 0 = q, 1 = k
                for c in range(4):
                    emit_proj_group(0, ti, c, "scalar")
            while rope_pend:
                rope_tail()

            if DBG:
                nc.sync.dma_start(d_qT[:], qT[0][:])
                nc.sync.dma_start(d_kT[:], kT[0][:])

            # ---------------- phase 2+3: attention g0 with fillers ----------------
            # ch0 fillers: v projection (sk-tile st emitted at sk=st so that
            # attnv(sp) finds v_dr[:, sp] ready)
            attention_chunk(0, 0, [
                (lambda st=st: emit_v_group(st)) for st in range(N_SK)
            ])
            # ch1..3 fillers: pair-1 projection + rope (8 groups + tail)
            g1_work = []
            for ti in range(2):
                for c in range(4):
                    g1_work.append(lambda ti=ti, c=c: emit_proj_group(1, ti, c, "vector"))
            g1_work.append(lambda: rope_pend and rope_tail())
            attention_chunk(0, 1, g1_work[0:3])
            attention_chunk(0, 2, g1_work[3:6])
            attention_chunk(0, 3, g1_work[6:9])
            while rope_pend:
                rope_tail()

            # ---------------- phase 4: attention g1 with outproj fillers --------
            attention_chunk(1, 0, [])
            for ch in range(1, N_CH):
                # outproj for chunk ch-1 (att_o ready for both pairs)
                opw = []
                for sti in range(4):
                    st = 4 * (ch - 1) + sti
                    for n in range(2):
                        eng = "vector" if (sti + n) % 2 == 0 else "gpsimd"
                        opw.append(lambda st=st, n=n, eng=eng: emit_outproj_tile(st, n, eng))
                attention_chunk(1, ch, opw)
            for sti in range(4):
                st = 12 + sti
                for n in range(2):
                    eng = "vector" if (sti + n) % 2 == 0 else "gpsimd"
                    emit_outproj_tile(st, n, eng)
            if DBG:
                nc.sync.dma_start(d_v[:], v_c.rearrange("p a b c -> p (a b c)"))
                for g in range(PAIRS):
                    nc.sync.dma_start(d_ao[:, S * g : S * (g + 1)], att_o[g][:])

    nc.compile()
    return nc


def _get_program():
    if "nc" not in _BUILT:
        _BUILT["nc"] = _build_program()
    return _BUILT["nc"]


def _host_inputs(x, W_qkv, W_out):
    """Build the 8 per-core input maps."""
    import ml_dtypes

    f = np.float32
    bf = ml_dtypes.bfloat16
    x = np.asarray(x, dtype=f)
    W_qkv = np.asarray(W_qkv, dtype=f)
    W_out = np.asarray(W_out, dtype=f)

    inv_freq = 1.0 / (ROPE_THETA ** (np.arange(0, D, 2, dtype=np.float64) / D))
    p = np.arange(128)
    freq_row = inv_freq[(p % D) // 2]  # [128]
    ang = freq_row[:, None] * np.arange(S, dtype=np.float64)[None, :]  # [128, S]
    cos_t = np.cos(ang).astype(f)
    sign = np.where(p % 2 == 0, -1.0, 1.0)[:, None]
    sin_t = (np.sin(ang) * sign).astype(f)

    msw = np.zeros((128, 128), dtype=f)
    msw[p, p ^ 1] = 1.0

    maps = []
    for core in range(N_CORES):
        b, hg = divmod(core, HG)
        hs = [HPG * hg + i for i in range(HPG)]
        w_qk = np.concatenate(
            [W_qkv[:, h * D : (h + 1) * D] for h in hs]
            + [W_qkv[:, ATT + h * D : ATT + (h + 1) * D] for h in hs],
            axis=1,
        )
        w_v = np.concatenate(
            [W_qkv[:, 2 * ATT + h * D : 2 * ATT + (h + 1) * D] for h in hs], axis=1
        )
        w_o = np.concatenate([W_out[h * D : (h + 1) * D, :] for h in hs], axis=0)
        maps.append(
            {
                "xT": np.ascontiguousarray(x[b].T).astype(bf),
                "w_qk": np.ascontiguousarray(w_qk).astype(bf),
                "w_v": np.ascontiguousarray(w_v).astype(bf),
                "w_o": np.ascontiguousarray(w_o).astype(bf),
                "cos_t": cos_t,
                "sin_t": sin_t,
                "mswap": msw.astype(bf),
            }
        )
    return maps


def kernel(x, W_qkv, W_out):
    from concourse.bass_utils import run_bass_kernel_spmd

    nc = _get_program()
    maps = _host_inputs(x, W_qkv, W_out)
    res = run_bass_kernel_spmd(nc, maps, core_ids=list(range(N_CORES)))
    out = np.zeros((B, S, E), dtype=np.float32)
    for core in range(N_CORES):
        b = core // HG
        out[b] += np.asarray(res.results[core]["out"], dtype=np.float32)
    return out


# revision 20
# speedup vs baseline: 1.0048x; 1.0048x over previous
"""Trainium2 Bass kernel for MultiHeadSelfAttention with RoPE.

Problem: x[2, 2048, 1024] @ W_qkv[1024, 3072] -> rope(q,k) -> softmax(q k^T/8) v
         -> out @ W_out[1024, 1024].

Sharding (8 cores): batch (2-way) x head-group (4-way, 4 heads each).
Each core computes a partial output [2048, 1024] = attnout_heads @ W_out_rows;
host sums the 4 head-group partials per batch.

v2 design (vs v1):
  - scores: two concurrent K=64 matmuls via PE row-tiling (head A on array
    rows 0:63, head B on rows 64:127, auto tile_position from base
    partitions) instead of zero-padded K=128 -> ~2x PE scores time.
  - attn@v: fp8e4 DoubleRow matmuls (2 sk-tiles = K=256 virtual per MM at
    0.5 cyc/row) with the softmax-denominator ones-column aug (M=65).
    exp() writes fp8e4 directly from PSUM scores.
  - exp: one ACT instr per (sk, both heads) = [128, 1024], ping-pong on a
    4-bank PSUM scores tile so scores(sk+2) overlap exp(sk).
  - bf16 everywhere else (q/k/att_o/W_out); f32 PSUM accumulation.
  - in-order engine queues: emission order interleaves v-proj into
    attention(g0,ch0), pair-1 projection into attention(g0,ch1..3), and the
    output projection into attention(g1), so everything hides under the
    ScalarE exp stream (the bottleneck at ~1.15us per [128,1024] tile).
"""

import sys

if "/opt/trn_rl_repo" not in sys.path:
    sys.path.insert(0, "/opt/trn_rl_repo")

import numpy as np

B, S, E = 2, 2048, 1024
ATT = 1024
H = 16
D = 64
HG = 4            # head groups (cores per batch)
HPG = H // HG     # heads per core = 4
PAIRS = HPG // 2  # head pairs per core = 2
ROPE_THETA = 10000.0
N_CORES = 8

SQ = 512          # sq chunk
N_CH = S // SQ    # 4 chunks
N_SK = S // 128   # 16 sk tiles
N_SP = N_SK // 2  # 8 sk-tile pairs
EK = E // 128     # 8 contraction tiles over embedding dim
# -ln(64): numerator and denominator share the shift so softmax is unchanged.
# Keeps fp8e4 exp below 448 up to score 82 (observed max 62.3); drops only
# ~0.1% of softmax mass into the subnormal/zero range.
EXP_BIAS = -4.1588830833596715
# Schraudolph fast-exp (for the sk tiles offloaded from ScalarE to DVE):
# i32 = int(A*s + B); bitcast(i32) ~ exp(0.125*s + EXP_BIAS) * (1 +- 1.8% rms).
# A = 0.125 * 2^23/ln2; B = 127*2^23 + EXP_BIAS*2^23/ln2 - C, C=482804
# calibrated on hardware (mini_test2) to zero the mean log error so
# Schraudolph'd softmax weights are unbiased vs the ACT-exp'd ones.
SCH_A = 0.125 * 12102203.161561485
SCH_B = 1065353216.0 - 50331648.0 - 482804.0

_BUILT = {}
DBG = False


def _build_program():
    import concourse.bacc as bacc
    import concourse.tile as tile
    import concourse.mybir as mybir

    f32 = mybir.dt.float32
    bf16 = mybir.dt.bfloat16
    i32 = mybir.dt.int32
    AF = mybir.ActivationFunctionType
    ALU = mybir.AluOpType

    nc = bacc.Bacc(
        "TRN2",
        target_bir_lowering=False,
        debug=False,
        enable_asserts=False,
        num_devices=N_CORES,
    )

    xT = nc.dram_tensor("xT", [E, S], bf16, kind="ExternalInput").ap()
    w_qk = nc.dram_tensor("w_qk", [E, 2 * HPG * D], bf16, kind="ExternalInput").ap()
    w_v = nc.dram_tensor("w_v", [E, HPG * D], bf16, kind="ExternalInput").ap()
    w_o = nc.dram_tensor("w_o", [HPG * D, E], bf16, kind="ExternalInput").ap()
    cos_t = nc.dram_tensor("cos_t", [128, S], f32, kind="ExternalInput").ap()
    sin_t = nc.dram_tensor("sin_t", [128, S], f32, kind="ExternalInput").ap()
    mswap = nc.dram_tensor("mswap", [128, 128], bf16, kind="ExternalInput").ap()
    out = nc.dram_tensor("out", [S, E], bf16, kind="ExternalOutput").ap()
    if DBG:
        d_qT = nc.dram_tensor("d_qT", [128, S], bf16, kind="ExternalOutput").ap()
        d_kT = nc.dram_tensor("d_kT", [128, S], bf16, kind="ExternalOutput").ap()
        d_v = nc.dram_tensor("d_v", [128, N_SK * HPG * 65], bf16, kind="ExternalOutput").ap()
        d_e = nc.dram_tensor("d_e", [128, 2 * SQ], bf16, kind="ExternalOutput").ap()
        d_oT = nc.dram_tensor("d_oT", [65, 2 * SQ], f32, kind="ExternalOutput").ap()
        d_ao = nc.dram_tensor("d_ao", [128, 2 * S], bf16, kind="ExternalOutput").ap()

    with tile.TileContext(nc) as tc:
        with (
            tc.tile_pool(name="const", bufs=1) as constp,
            tc.tile_pool(name="xt", bufs=1) as xtp,
            tc.tile_pool(name="wqk", bufs=1) as wqkp,
            tc.tile_pool(name="trig", bufs=1) as trigp,
            tc.tile_pool(name="qkT", bufs=1) as qkTp,
            tc.tile_pool(name="vdr", bufs=1) as vp,
            tc.tile_pool(name="attnout", bufs=1) as aop,
            tc.tile_pool(name="wo", bufs=1) as wop,
            tc.tile_pool(name="ropes", bufs=2) as ropep,
            tc.tile_pool(name="exps", bufs=1) as expp,
            tc.tile_pool(name="norm", bufs=2) as rcp,
            tc.tile_pool(name="osb", bufs=3) as osbp,
            tc.tile_pool(name="ps", bufs=1, space="PSUM") as psp,
        ):
            # ---------------- static tiles ----------------
            msw_sb = constp.tile([128, 128], bf16, tag="msw")
            onesrow = constp.tile([65, 64], bf16, tag="onesrow")
            bias_t = constp.tile([128, 1], f32, tag="bias")
            nc.gpsimd.memset(onesrow[64:65, :], 1.0)
            nc.gpsimd.memset(bias_t[:], EXP_BIAS)

            # q'/k' per pair: [128, S] bf16; rows 0:64 head A dims, 64:128 head B
            qT = [qkTp.tile([128, S], bf16, tag=f"qT{g}", name=f"qT{g}") for g in range(PAIRS)]
            kT = [qkTp.tile([128, S], bf16, tag=f"kT{g}", name=f"kT{g}") for g in range(PAIRS)]
            # v in bf16: [ki, st, head, 65]; cols 0:64 = v dims, col 64 = ones
            # (softmax-denominator aug: oT row 64 accumulates sum of weights)
            v_c = vp.tile([128, N_SK, HPG, 65], bf16, tag="vc")
            nc.gpsimd.memset(v_c[:, :, :, 64:65], 1.0)
            # normalized attention output per pair [128 (pair dims), S] bf16
            att_o = [aop.tile([128, S], bf16, tag=f"ao{g}", name=f"ao{g}") for g in range(PAIRS)]
            wo_sb = [wop.tile([128, E], bf16, tag=f"wo{g}", name=f"wo{g}") for g in range(PAIRS)]

            cos_sb = trigp.tile([128, S], f32, tag="cos")
            sin_sb = trigp.tile([128, S], f32, tag="sin")
            wqk_all = wqkp.tile([128, EK, 2 * HPG * D], bf16, tag="wqk")
            wv_all = wqkp.tile([128, EK, HPG * D], bf16, tag="wv")
            xt_all = xtp.tile([128, EK, S], bf16, tag="xt")

            # ---------------- DMA (consumption order) ----------------
            nc.sync.dma_start(msw_sb[:], mswap[:])
            wqk_d = w_qk.rearrange("(ek p) c -> p ek c", p=128)
            xt_d = xT.rearrange("(ek p) s -> p ek s", p=128)
            nc.sync.dma_start(wqk_all[:], wqk_d)
            nc.sync.dma_start(xt_all[:, :, 0:512], xt_d[:, :, 0:512])
            nc.sync.dma_start(cos_sb[:, 0:512], cos_t[:, 0:512])
            nc.sync.dma_start(sin_sb[:, 0:512], sin_t[:, 0:512])
            for c in range(1, 4):
                csl = slice(512 * c, 512 * (c + 1))
                nc.sync.dma_start(xt_all[:, :, csl], xt_d[:, :, csl])
                nc.sync.dma_start(cos_sb[:, csl], cos_t[:, csl])
                nc.sync.dma_start(sin_sb[:, csl], sin_t[:, csl])
            nc.sync.dma_start(wv_all[:], w_v.rearrange("(ek p) c -> p ek c", p=128))
            for g in range(PAIRS):
                nc.sync.dma_start(wo_sb[g][:], w_o[128 * g : 128 * (g + 1), :])

            wqk_sb = [wqk_all[:, e, :] for e in range(EK)]
            xt_sb = [xt_all[:, e, :] for e in range(EK)]
            wv_sb = [wv_all[:, e, :] for e in range(EK)]

            # ---------------- helpers ----------------
            rope_pend = []

            def emit_proj_group(g, ti, c, evac_eng):
                """One 512-col chunk of the q/k projection for pair g."""
                sl = slice(512 * c, 512 * (c + 1))
                coff = ti * HPG * D + 128 * g
                pp = psp.tile([128, 512], f32, tag="pj", bufs=2, name="pp")
                for e in range(EK):
                    nc.tensor.matmul(
                        pp[:],
                        wqk_sb[e][:, coff : coff + 128],
                        xt_sb[e][:, sl],
                        start=(e == 0),
                        stop=(e == EK - 1),
                    )
                raw = ropep.tile([128, 512], bf16, tag="raw", name="raw")
                if evac_eng == "scalar":
                    nc.scalar.copy(raw[:], pp[:])
                else:
                    nc.vector.tensor_copy(raw[:], pp[:])
                dest = (qT, kT)[ti][g]
                rope_pend.append((dest, sl, raw))
                if len(rope_pend) > 1:
                    rope_tail()

            def rope_tail():
                (dest, sl, raw) = rope_pend.pop(0)
                rp = psp.tile([128, 512], f32, tag="pj", bufs=2, name="rp")
                nc.tensor.matmul(rp[:], msw_sb[:], raw[:], start=True, stop=True)
                t2 = ropep.tile([128, 512], f32, tag="t2")
                nc.vector.tensor_mul(t2[:], raw[:], cos_sb[:, sl])
                t1 = ropep.tile([128, 512], f32, tag="t1")
                nc.vector.tensor_mul(t1[:], rp[:], sin_sb[:, sl])
                nc.gpsimd.tensor_tensor(dest[:, sl], t1[:], t2[:], ALU.add)

            def emit_v_group(st):
                """v projection for sk-tile st (all 4 heads), into v_c bf16."""
                vp_ps = psp.tile([128, HPG * D], f32, tag="pj", bufs=2, name="vps")
                for e in range(EK):
                    nc.tensor.matmul(
                        vp_ps[:],
                        xt_sb[e][:, 128 * st : 128 * (st + 1)],
                        wv_sb[e][:],
                        start=(e == 0),
                        stop=(e == EK - 1),
                    )
                nc.vector.tensor_copy(
                    v_c[:, st, :, 0:64],
                    vp_ps.rearrange("p (h c) -> p h c", h=HPG),
                )

            def emit_outproj_tile(st, n, eng):
                """out[st*128:(st+1)*128, n*512:(n+1)*512] = attn_out @ W_out."""
                ssl = slice(128 * st, 128 * (st + 1))
                nsl = slice(512 * n, 512 * (n + 1))
                op = psp.tile([128, 512], f32, tag="pj", bufs=2, name="op")
                for g in range(PAIRS):
                    nc.tensor.matmul(
                        op[:],
                        att_o[g][:, ssl],
                        wo_sb[g][:, nsl],
                        start=(g == 0),
                        stop=(g == PAIRS - 1),
                    )
                ot = osbp.tile([128, 512], bf16, tag="ot")
                nc.vector.tensor_copy(ot[:], op[:])
                nc.sync.dma_start(out[ssl, nsl], ot[:])

            def attention_chunk(g, ch, fillers):
                csl = slice(SQ * ch, SQ * (ch + 1))
                sps = psp.tile([128, 2, 2, SQ], f32, tag="sps", name="sps")
                oT = [
                    psp.tile([65, SQ], f32, tag=f"oT{h}", name=f"oT{h}")
                    for h in range(2)
                ]
                e_cur = None
                fi = 0
                for sk in range(N_SK):
                    par = sk % 2
                    sksl = slice(128 * sk, 128 * (sk + 1))
                    for h in range(2):
                        pb = 64 * h
                        nc.tensor.matmul(
                            sps[:, par, h, :],
                            kT[g][pb : pb + 64, sksl],
                            qT[g][pb : pb + 64, csl],
                            start=True,
                            stop=True,
                        )
                    e_cur = expp.tile([128, 2, SQ], bf16, tag="e", bufs=2, name="ecur")
                    if sk % 4 == 3:
                        # Schraudolph exp off the ScalarE critical path:
                        # DVE does the PSUM read + int math (also releases the
                        # scores WAR early); gpsimd converts bitcast->bf16.
                        ei = expp.tile([128, 2, SQ], i32, tag="ei", bufs=2, name="ei")
                        nc.vector.tensor_scalar(
                            ei[:], sps[:, par, :, :], SCH_A, SCH_B,
                            ALU.mult, ALU.add,
                        )
                        nc.gpsimd.tensor_scalar(
                            e_cur[:], ei.bitcast(f32), 1.0, 0.0, ALU.mult, ALU.add
                        )
                    else:
                        nc.scalar.activation(
                            e_cur[:],
                            sps[:, par, :, :],
                            AF.Exp,
                            scale=0.125,
                            bias=bias_t[:],
                        )
                    if fi < len(fillers) and sk % (N_SK // len(fillers)) == (N_SK // len(fillers)) - 1:
                        fillers[fi]()
                        fi += 1
                    if DBG and g == 0 and ch == 0 and sk == 0:
                        nc.sync.dma_start(d_e[:], e_cur.rearrange("p a b -> p (a b)"))
                    for h in range(2):
                        hh = 2 * g + h
                        nc.tensor.matmul(
                            oT[h][:],
                            v_c[:, sk, hh, 0:65],
                            e_cur[:, h, :],
                            start=(sk == 0),
                            stop=(sk == N_SK - 1),
                        )
                while fi < len(fillers):
                    fillers[fi]()
                    fi += 1
                # ---- normalize: row 64 of oT holds the denominators ----
                if DBG and g == 0 and ch == 0:
                    for h in range(2):
                        dt_ = rcp.tile([65, SQ], f32, tag="dbg", name="dbgt")
                        nc.vector.tensor_copy(dt_[:], oT[h][:])
                        nc.sync.dma_start(d_oT[:, SQ * h : SQ * (h + 1)], dt_[:])
                for h in range(2):
                    o_s = rcp.tile([65, SQ], bf16, tag=f"o{h}", name=f"os{h}")
                    nc.vector.tensor_copy(o_s[:], oT[h][:])
                    db = psp.tile([64, SQ], f32, tag=f"oT{h}", name=f"db{h}")
                    nc.tensor.matmul(
                        db[:], onesrow[64:65, :], o_s[64:65, :], start=True, stop=True
                    )
                    rb = rcp.tile([64, SQ], f32, tag=f"rb{h}", name=f"rb{h}")
                    nc.vector.reciprocal_approx_fast(rb[:], db[:])
                    if h == 0:
                        nc.vector.tensor_mul(
                            att_o[g][0:64, csl], o_s[0:64, :], rb[:]
                        )
                    else:
                        aoB = rcp.tile([64, SQ], bf16, tag="aoB", name="aoB")
                        nc.vector.tensor_mul(aoB[:], o_s[0:64, :], rb[:])
                        nc.sync.dma_start(att_o[g][64:128, csl], aoB[:])

            # ---------------- phase 1: pair-0 projection + rope ----------------
            for ti in range(2):          # 0 = q, 1 = k
                for c in range(4):
                    emit_proj_group(0, ti, c, "scalar")
            while rope_pend:
                rope_tail()

            if DBG:
                nc.sync.dma_start(d_qT[:], qT[0][:])
                nc.sync.dma_start(d_kT[:], kT[0][:])

            # ---------------- phase 2+3: attention g0 with fillers ----------------
            # ch0 fillers: v projection (sk-tile st emitted at sk=st so that
            # attnv(sp) finds v_dr[:, sp] ready)
            attention_chunk(0, 0, [
                (lambda st=st: emit_v_group(st)) for st in range(N_SK)
            ])
            # ch1..3 fillers: pair-1 projection + rope (8 groups + tail)
            g1_work = []
            for ti in range(2):
                for c in range(4):
                    g1_work.append(lambda ti=ti, c=c: emit_proj_group(1, ti, c, "vector"))
            g1_work.append(lambda: rope_pend and rope_tail())
            attention_chunk(0, 1, g1_work[0:3])
            attention_chunk(0, 2, g1_work[3:6])
            attention_chunk(0, 3, g1_work[6:9])
            while rope_pend:
                rope_tail()

            # ---------------- phase 4: attention g1 with outproj fillers --------
            attention_chunk(1, 0, [])
            for ch in range(1, N_CH):
                # outproj for chunk ch-1 (att_o ready for both pairs)
                opw = []
                for sti in range(4):
                    st = 4 * (ch - 1) + sti
                    for n in range(2):
                        eng = "vector" if (sti + n) % 2 == 0 else "gpsimd"
                        opw.append(lambda st=st, n=n, eng=eng: emit_outproj_tile(st, n, eng))
                attention_chunk(1, ch, opw)
            for sti in range(4):
                st = 12 + sti
                for n in range(2):
                    eng = "vector" if (sti + n) % 2 == 0 else "gpsimd"
                    emit_outproj_tile(st, n, eng)
            if DBG:
                nc.sync.dma_start(d_v[:], v_c.rearrange("p a b c -> p (a b c)"))
                for g in range(PAIRS):
                    nc.sync.dma_start(d_ao[:, S * g : S * (g + 1)], att_o[g][:])

    nc.compile()
    return nc


def _get_program():
    if "nc" not in _BUILT:
        _BUILT["nc"] = _build_program()
    return _BUILT["nc"]


def _host_inputs(x, W_qkv, W_out):
    """Build the 8 per-core input maps."""
    import ml_dtypes

    f = np.float32
    bf = ml_dtypes.bfloat16
    x = np.asarray(x, dtype=f)
    W_qkv = np.asarray(W_qkv, dtype=f)
    W_out = np.asarray(W_out, dtype=f)

    inv_freq = 1.0 / (ROPE_THETA ** (np.arange(0, D, 2, dtype=np.float64) / D))
    p = np.arange(128)
    freq_row = inv_freq[(p % D) // 2]  # [128]
    ang = freq_row[:, None] * np.arange(S, dtype=np.float64)[None, :]  # [128, S]
    cos_t = np.cos(ang).astype(f)
    sign = np.where(p % 2 == 0, -1.0, 1.0)[:, None]
    sin_t = (np.sin(ang) * sign).astype(f)

    msw = np.zeros((128, 128), dtype=f)
    msw[p, p ^ 1] = 1.0

    maps = []
    for core in range(N_CORES):
        b, hg = divmod(core, HG)
        hs = [HPG * hg + i for i in range(HPG)]
        w_qk = np.concatenate(
            [W_qkv[:, h * D : (h + 1) * D] for h in hs]
            + [W_qkv[:, ATT + h * D : ATT + (h + 1) * D] for h in hs],
            axis=1,
        )
        w_v = np.concatenate(
            [W_qkv[:, 2 * ATT + h * D : 2 * ATT + (h + 1) * D] for h in hs], axis=1
        )
        w_o = np.concatenate([W_out[h * D : (h + 1) * D, :] for h in hs], axis=0)
        maps.append(
            {
                "xT": np.ascontiguousarray(x[b].T).astype(bf),
                "w_qk": np.ascontiguousarray(w_qk).astype(bf),
                "w_v": np.ascontiguousarray(w_v).astype(bf),
                "w_o": np.ascontiguousarray(w_o).astype(bf),
                "cos_t": cos_t,
                "sin_t": sin_t,
                "mswap": msw.astype(bf),
            }
        )
    return maps


def kernel(x, W_qkv, W_out):
    from concourse.bass_utils import run_bass_kernel_spmd

    nc = _get_program()
    maps = _host_inputs(x, W_qkv, W_out)
    res = run_bass_kernel_spmd(nc, maps, core_ids=list(range(N_CORES)))
    out = np.zeros((B, S, E), dtype=np.float32)
    for core in range(N_CORES):
        b = core // HG
        out[b] += np.asarray(res.results[core]["out"], dtype=np.float32)
    return out


# revision 22
# speedup vs baseline: 1.1968x; 1.1911x over previous
"""Trainium2 Bass kernel for MultiHeadSelfAttention with RoPE.

Problem: x[2, 2048, 1024] @ W_qkv[1024, 3072] -> rope(q,k) -> softmax(q k^T/8) v
         -> out @ W_out[1024, 1024].

Sharding (8 cores): batch (2-way) x head-group (4-way, 4 heads each).
Each core computes a partial output [2048, 1024] = attnout_heads @ W_out_rows;
host sums the 4 head-group partials per batch.

v2 design (vs v1):
  - scores: two concurrent K=64 matmuls via PE row-tiling (head A on array
    rows 0:63, head B on rows 64:127, auto tile_position from base
    partitions) instead of zero-padded K=128 -> ~2x PE scores time.
  - attn@v: fp8e4 DoubleRow matmuls (2 sk-tiles = K=256 virtual per MM at
    0.5 cyc/row) with the softmax-denominator ones-column aug (M=65).
    exp() writes fp8e4 directly from PSUM scores.
  - exp: one ACT instr per (sk, both heads) = [128, 1024], ping-pong on a
    4-bank PSUM scores tile so scores(sk+2) overlap exp(sk).
  - bf16 everywhere else (q/k/att_o/W_out); f32 PSUM accumulation.
  - in-order engine queues: emission order interleaves v-proj into
    attention(g0,ch0), pair-1 projection into attention(g0,ch1..3), and the
    output projection into attention(g1), so everything hides under the
    ScalarE exp stream (the bottleneck at ~1.15us per [128,1024] tile).
"""

import sys

if "/opt/trn_rl_repo" not in sys.path:
    sys.path.insert(0, "/opt/trn_rl_repo")

import numpy as np

B, S, E = 2, 2048, 1024
ATT = 1024
H = 16
D = 64
HG = 4            # head groups (cores per batch)
HPG = H // HG     # heads per core = 4
PAIRS = HPG // 2  # head pairs per core = 2
ROPE_THETA = 10000.0
N_CORES = 8

SQ = 512          # sq chunk
N_CH = S // SQ    # 4 chunks
N_SK = S // 128   # 16 sk tiles
N_SP = N_SK // 2  # 8 sk-tile pairs
EK = E // 128     # 8 contraction tiles over embedding dim
# -ln(64): numerator and denominator share the shift so softmax is unchanged.
# Keeps fp8e4 exp below 448 up to score 82 (observed max 62.3); drops only
# ~0.1% of softmax mass into the subnormal/zero range.
EXP_BIAS = -4.1588830833596715
# Schraudolph fast-exp (for the sk tiles offloaded from ScalarE to DVE):
# i32 = int(A*s + B); bitcast(i32) ~ exp(0.125*s + EXP_BIAS) * (1 +- 1.8% rms).
# A = 0.125 * 2^23/ln2; B = 127*2^23 + EXP_BIAS*2^23/ln2 - C, C=482804
# calibrated on hardware (mini_test2) to zero the mean log error so
# Schraudolph'd softmax weights are unbiased vs the ACT-exp'd ones.
SCH_A = 0.125 * 12102203.161561485
SCH_B = 1065353216.0 - 50331648.0 - 482804.0

_BUILT = {}
DBG = False


def _build_program():
    import concourse.bacc as bacc
    import concourse.tile as tile
    import concourse.mybir as mybir

    f32 = mybir.dt.float32
    bf16 = mybir.dt.bfloat16
    i32 = mybir.dt.int32
    AF = mybir.ActivationFunctionType
    ALU = mybir.AluOpType

    nc = bacc.Bacc(
        "TRN2",
        target_bir_lowering=False,
        debug=False,
        enable_asserts=False,
        num_devices=N_CORES,
    )

    xT = nc.dram_tensor("xT", [E, S], bf16, kind="ExternalInput").ap()
    w_qk = nc.dram_tensor("w_qk", [E, 2 * HPG * D], bf16, kind="ExternalInput").ap()
    w_v = nc.dram_tensor("w_v", [E, HPG * D], bf16, kind="ExternalInput").ap()
    w_o = nc.dram_tensor("w_o", [HPG * D, E], bf16, kind="ExternalInput").ap()
    cos_t = nc.dram_tensor("cos_t", [128, S], f32, kind="ExternalInput").ap()
    sin_t = nc.dram_tensor("sin_t", [128, S], f32, kind="ExternalInput").ap()
    mswap = nc.dram_tensor("mswap", [128, 128], bf16, kind="ExternalInput").ap()
    out = nc.dram_tensor("out", [S, E], bf16, kind="ExternalOutput").ap()
    if DBG:
        d_qT = nc.dram_tensor("d_qT", [128, S], bf16, kind="ExternalOutput").ap()
        d_kT = nc.dram_tensor("d_kT", [128, S], bf16, kind="ExternalOutput").ap()
        d_v = nc.dram_tensor("d_v", [128, N_SK * HPG * 65], bf16, kind="ExternalOutput").ap()
        d_e = nc.dram_tensor("d_e", [128, 2 * SQ], bf16, kind="ExternalOutput").ap()
        d_oT = nc.dram_tensor("d_oT", [65, 2 * SQ], f32, kind="ExternalOutput").ap()
        d_ao = nc.dram_tensor("d_ao", [128, 2 * S], bf16, kind="ExternalOutput").ap()

    with tile.TileContext(nc) as tc:
        with (
            tc.tile_pool(name="const", bufs=1) as constp,
            tc.tile_pool(name="xt", bufs=1) as xtp,
            tc.tile_pool(name="wqk", bufs=1) as wqkp,
            tc.tile_pool(name="trig", bufs=1) as trigp,
            tc.tile_pool(name="qkT", bufs=1) as qkTp,
            tc.tile_pool(name="vdr", bufs=1) as vp,
            tc.tile_pool(name="attnout", bufs=1) as aop,
            tc.tile_pool(name="wo", bufs=1) as wop,
            tc.tile_pool(name="ropes", bufs=2) as ropep,
            tc.tile_pool(name="exps", bufs=1) as expp,
            tc.tile_pool(name="norm", bufs=2) as rcp,
            tc.tile_pool(name="osb", bufs=3) as osbp,
            tc.tile_pool(name="ps", bufs=1, space="PSUM") as psp,
        ):
            # ---------------- static tiles ----------------
            msw_sb = constp.tile([128, 128], bf16, tag="msw")
            onesrow = constp.tile([65, 64], bf16, tag="onesrow")
            bias_t = constp.tile([128, 1], f32, tag="bias")
            nc.gpsimd.memset(onesrow[64:65, :], 1.0)
            nc.gpsimd.memset(bias_t[:], EXP_BIAS)

            # q'/k' per pair: [128, S] bf16; rows 0:64 head A dims, 64:128 head B
            qT = [qkTp.tile([128, S], bf16, tag=f"qT{g}", name=f"qT{g}") for g in range(PAIRS)]
            kT = [qkTp.tile([128, S], bf16, tag=f"kT{g}", name=f"kT{g}") for g in range(PAIRS)]
            # v in bf16: [ki, st, head, 65]; cols 0:64 = v dims, col 64 = ones
            # (softmax-denominator aug: oT row 64 accumulates sum of weights)
            v_c = vp.tile([128, N_SK, HPG, 65], bf16, tag="vc")
            nc.gpsimd.memset(v_c[:, :, :, 64:65], 1.0)
            # normalized attention output per pair [128 (pair dims), S] bf16
            att_o = [aop.tile([128, S], bf16, tag=f"ao{g}", name=f"ao{g}") for g in range(PAIRS)]
            wo_sb = [wop.tile([128, E], bf16, tag=f"wo{g}", name=f"wo{g}") for g in range(PAIRS)]

            cos_sb = trigp.tile([128, S], f32, tag="cos")
            sin_sb = trigp.tile([128, S], f32, tag="sin")
            wqk_all = wqkp.tile([128, EK, 2 * HPG * D], bf16, tag="wqk")
            wv_all = wqkp.tile([128, EK, HPG * D], bf16, tag="wv")
            xt_all = xtp.tile([128, EK, S], bf16, tag="xt")

            # ---------------- DMA (consumption order) ----------------
            nc.sync.dma_start(msw_sb[:], mswap[:])
            wqk_d = w_qk.rearrange("(ek p) c -> p ek c", p=128)
            xt_d = xT.rearrange("(ek p) s -> p ek s", p=128)
            nc.sync.dma_start(wqk_all[:], wqk_d)
            nc.sync.dma_start(xt_all[:, :, 0:512], xt_d[:, :, 0:512])
            nc.sync.dma_start(cos_sb[:, 0:512], cos_t[:, 0:512])
            nc.sync.dma_start(sin_sb[:, 0:512], sin_t[:, 0:512])
            for c in range(1, 4):
                csl = slice(512 * c, 512 * (c + 1))
                nc.sync.dma_start(xt_all[:, :, csl], xt_d[:, :, csl])
                nc.sync.dma_start(cos_sb[:, csl], cos_t[:, csl])
                nc.sync.dma_start(sin_sb[:, csl], sin_t[:, csl])
            nc.sync.dma_start(wv_all[:], w_v.rearrange("(ek p) c -> p ek c", p=128))
            for g in range(PAIRS):
                nc.sync.dma_start(wo_sb[g][:], w_o[128 * g : 128 * (g + 1), :])

            wqk_sb = [wqk_all[:, e, :] for e in range(EK)]
            xt_sb = [xt_all[:, e, :] for e in range(EK)]
            wv_sb = [wv_all[:, e, :] for e in range(EK)]

            # ---------------- helpers ----------------
            rope_pend = []

            def emit_proj_group(g, ti, c, evac_eng):
                """One 512-col chunk of the q/k projection for pair g."""
                sl = slice(512 * c, 512 * (c + 1))
                coff = ti * HPG * D + 128 * g
                pp = psp.tile([128, 512], f32, tag="pj", bufs=2, name="pp")
                for e in range(EK):
                    nc.tensor.matmul(
                        pp[:],
                        wqk_sb[e][:, coff : coff + 128],
                        xt_sb[e][:, sl],
                        start=(e == 0),
                        stop=(e == EK - 1),
                    )
                raw = ropep.tile([128, 512], bf16, tag="raw", name="raw")
                if evac_eng == "scalar":
                    nc.scalar.copy(raw[:], pp[:])
                else:
                    nc.vector.tensor_copy(raw[:], pp[:])
                dest = (qT, kT)[ti][g]
                rope_pend.append((dest, sl, raw))
                if len(rope_pend) > 1:
                    rope_tail()

            def rope_tail():
                (dest, sl, raw) = rope_pend.pop(0)
                rp = psp.tile([128, 512], f32, tag="pj", bufs=2, name="rp")
                nc.tensor.matmul(rp[:], msw_sb[:], raw[:], start=True, stop=True)
                t2 = ropep.tile([128, 512], f32, tag="t2")
                nc.vector.tensor_mul(t2[:], raw[:], cos_sb[:, sl])
                t1 = ropep.tile([128, 512], f32, tag="t1")
                nc.vector.tensor_mul(t1[:], rp[:], sin_sb[:, sl])
                nc.gpsimd.tensor_tensor(dest[:, sl], t1[:], t2[:], ALU.add)

            def emit_v_group(st):
                """v projection for sk-tile st (all 4 heads), into v_c bf16."""
                vp_ps = psp.tile([128, HPG * D], f32, tag="pj", bufs=2, name="vps")
                for e in range(EK):
                    nc.tensor.matmul(
                        vp_ps[:],
                        xt_sb[e][:, 128 * st : 128 * (st + 1)],
                        wv_sb[e][:],
                        start=(e == 0),
                        stop=(e == EK - 1),
                    )
                nc.vector.tensor_copy(
                    v_c[:, st, :, 0:64],
                    vp_ps.rearrange("p (h c) -> p h c", h=HPG),
                )

            def emit_outproj_tile(st, n, eng):
                """out[st*128:(st+1)*128, n*512:(n+1)*512] = attn_out @ W_out."""
                ssl = slice(128 * st, 128 * (st + 1))
                nsl = slice(512 * n, 512 * (n + 1))
                op = psp.tile([128, 512], f32, tag="pj", bufs=2, name="op")
                for g in range(PAIRS):
                    nc.tensor.matmul(
                        op[:],
                        att_o[g][:, ssl],
                        wo_sb[g][:, nsl],
                        start=(g == 0),
                        stop=(g == PAIRS - 1),
                    )
                ot = osbp.tile([128, 512], bf16, tag="ot")
                nc.vector.tensor_copy(ot[:], op[:])
                nc.sync.dma_start(out[ssl, nsl], ot[:])

            def heat(n):
                # Standalone LDWEIGHTS as a PE "heater": keeps the PE busy
                # through producer-chain waits so the HAM never re-throttles
                # the clock to 1.2 GHz. Harmless: every real matmul self-loads
                # its own weights.
                for _ in range(n):
                    nc.tensor.ldweights(wqk_all[:, 0, 0:128])

            def attnv(g, sk, e_t, oT):
                for h in range(2):
                    hh = 2 * g + h
                    nc.tensor.matmul(
                        oT[h][:],
                        v_c[:, sk, hh, 0:65],
                        e_t[:, h, :],
                        start=(sk == 0),
                        stop=(sk == N_SK - 1),
                    )

            def attention_chunk(g, ch, fillers):
                csl = slice(SQ * ch, SQ * (ch + 1))
                sps = psp.tile([128, 2, 2, SQ], f32, tag="sps", name="sps")
                oT = [
                    psp.tile([65, SQ], f32, tag=f"oT{h}", name=f"oT{h}")
                    for h in range(2)
                ]
                e_hist = []
                fi = 0
                for sk in range(N_SK):
                    par = sk % 2
                    sksl = slice(128 * sk, 128 * (sk + 1))
                    for h in range(2):
                        pb = 64 * h
                        nc.tensor.matmul(
                            sps[:, par, h, :],
                            kT[g][pb : pb + 64, sksl],
                            qT[g][pb : pb + 64, csl],
                            start=True,
                            stop=True,
                        )
                    e_cur = expp.tile([128, 2, SQ], bf16, tag="e", bufs=3, name="ecur")
                    if sk % 4 == 3:
                        # Schraudolph exp off the ScalarE critical path:
                        # DVE does the PSUM read + int math (also releases the
                        # scores WAR early); gpsimd converts bitcast->bf16.
                        ei = expp.tile([128, 2, SQ], i32, tag="ei", bufs=2, name="ei")
                        nc.vector.tensor_scalar(
                            ei[:], sps[:, par, :, :], SCH_A, SCH_B,
                            ALU.mult, ALU.add,
                        )
                        nc.gpsimd.tensor_scalar(
                            e_cur[:], ei.bitcast(f32), 1.0, 0.0, ALU.mult, ALU.add
                        )
                    else:
                        nc.scalar.activation(
                            e_cur[:],
                            sps[:, par, :, :],
                            AF.Exp,
                            scale=0.125,
                            bias=bias_t[:],
                        )
                    e_hist.append(e_cur)
                    if fi < len(fillers) and sk % (N_SK // len(fillers)) == (N_SK // len(fillers)) - 1:
                        fillers[fi]()
                        fi += 1
                    if DBG and g == 0 and ch == 0 and sk == 0:
                        nc.sync.dma_start(d_e[:], e_cur.rearrange("p a b -> p (a b)"))
                    # attn@v lags one sk so its exp input is already complete
                    if sk >= 1:
                        attnv(g, sk - 1, e_hist[sk - 1], oT)
                    heat(2)
                attnv(g, N_SK - 1, e_hist[N_SK - 1], oT)
                while fi < len(fillers):
                    fillers[fi]()
                    fi += 1
                # ---- normalize: row 64 of oT holds the denominators ----
                if DBG and g == 0 and ch == 0:
                    for h in range(2):
                        dt_ = rcp.tile([65, SQ], f32, tag="dbg", name="dbgt")
                        nc.vector.tensor_copy(dt_[:], oT[h][:])
                        nc.sync.dma_start(d_oT[:, SQ * h : SQ * (h + 1)], dt_[:])
                for h in range(2):
                    o_s = rcp.tile([65, SQ], bf16, tag=f"o{h}", name=f"os{h}")
                    nc.vector.tensor_copy(o_s[:], oT[h][:])
                    heat(3)
                    db = psp.tile([64, SQ], f32, tag=f"oT{h}", name=f"db{h}")
                    nc.tensor.matmul(
                        db[:], onesrow[64:65, :], o_s[64:65, :], start=True, stop=True
                    )
                    rb = rcp.tile([64, SQ], f32, tag=f"rb{h}", name=f"rb{h}")
                    nc.vector.reciprocal_approx_fast(rb[:], db[:])
                    if h == 0:
                        nc.vector.tensor_mul(
                            att_o[g][0:64, csl], o_s[0:64, :], rb[:]
                        )
                    else:
                        aoB = rcp.tile([64, SQ], bf16, tag="aoB", name="aoB")
                        nc.vector.tensor_mul(aoB[:], o_s[0:64, :], rb[:])
                        nc.sync.dma_start(att_o[g][64:128, csl], aoB[:])

            # ---------------- phase 1: pair-0 projection + rope ----------------
            for ti in range(2):          # 0 = q, 1 = k
                for c in range(4):
                    emit_proj_group(0, ti, c, "scalar")
            while rope_pend:
                rope_tail()

            if DBG:
                nc.sync.dma_start(d_qT[:], qT[0][:])
                nc.sync.dma_start(d_kT[:], kT[0][:])

            # ---------------- phase 2+3: attention g0 with fillers ----------------
            # ch0 fillers: v projection (sk-tile st emitted at sk=st so that
            # attnv(sp) finds v_dr[:, sp] ready)
            attention_chunk(0, 0, [
                (lambda st=st: emit_v_group(st)) for st in range(N_SK)
            ])
            # ch1..3 fillers: pair-1 projection + rope (8 groups + tail)
            g1_work = []
            for ti in range(2):
                for c in range(4):
                    g1_work.append(lambda ti=ti, c=c: emit_proj_group(1, ti, c, "vector"))
            g1_work.append(lambda: rope_pend and rope_tail())
            attention_chunk(0, 1, g1_work[0:3])
            attention_chunk(0, 2, g1_work[3:6])
            attention_chunk(0, 3, g1_work[6:9])
            while rope_pend:
                rope_tail()

            # ---------------- phase 4: attention g1 with outproj fillers --------
            attention_chunk(1, 0, [])
            for ch in range(1, N_CH):
                # outproj for chunk ch-1 (att_o ready for both pairs)
                opw = []
                for sti in range(4):
                    st = 4 * (ch - 1) + sti
                    for n in range(2):
                        eng = "vector" if (sti + n) % 2 == 0 else "gpsimd"
                        opw.append(lambda st=st, n=n, eng=eng: emit_outproj_tile(st, n, eng))
                attention_chunk(1, ch, opw)
            for sti in range(4):
                st = 12 + sti
                for n in range(2):
                    eng = "vector" if (sti + n) % 2 == 0 else "gpsimd"
                    emit_outproj_tile(st, n, eng)
            if DBG:
                nc.sync.dma_start(d_v[:], v_c.rearrange("p a b c -> p (a b c)"))
                for g in range(PAIRS):
                    nc.sync.dma_start(d_ao[:, S * g : S * (g + 1)], att_o[g][:])

    nc.compile()
    return nc


def _get_program():
    if "nc" not in _BUILT:
        _BUILT["nc"] = _build_program()
    return _BUILT["nc"]


def _host_inputs(x, W_qkv, W_out):
    """Build the 8 per-core input maps."""
    import ml_dtypes

    f = np.float32
    bf = ml_dtypes.bfloat16
    x = np.asarray(x, dtype=f)
    W_qkv = np.asarray(W_qkv, dtype=f)
    W_out = np.asarray(W_out, dtype=f)

    inv_freq = 1.0 / (ROPE_THETA ** (np.arange(0, D, 2, dtype=np.float64) / D))
    p = np.arange(128)
    freq_row = inv_freq[(p % D) // 2]  # [128]
    ang = freq_row[:, None] * np.arange(S, dtype=np.float64)[None, :]  # [128, S]
    cos_t = np.cos(ang).astype(f)
    sign = np.where(p % 2 == 0, -1.0, 1.0)[:, None]
    sin_t = (np.sin(ang) * sign).astype(f)

    msw = np.zeros((128, 128), dtype=f)
    msw[p, p ^ 1] = 1.0

    maps = []
    for core in range(N_CORES):
        b, hg = divmod(core, HG)
        hs = [HPG * hg + i for i in range(HPG)]
        w_qk = np.concatenate(
            [W_qkv[:, h * D : (h + 1) * D] for h in hs]
            + [W_qkv[:, ATT + h * D : ATT + (h + 1) * D] for h in hs],
            axis=1,
        )
        w_v = np.concatenate(
            [W_qkv[:, 2 * ATT + h * D : 2 * ATT + (h + 1) * D] for h in hs], axis=1
        )
        w_o = np.concatenate([W_out[h * D : (h + 1) * D, :] for h in hs], axis=0)
        maps.append(
            {
                "xT": np.ascontiguousarray(x[b].T).astype(bf),
                "w_qk": np.ascontiguousarray(w_qk).astype(bf),
                "w_v": np.ascontiguousarray(w_v).astype(bf),
                "w_o": np.ascontiguousarray(w_o).astype(bf),
                "cos_t": cos_t,
                "sin_t": sin_t,
                "mswap": msw.astype(bf),
            }
        )
    return maps


def kernel(x, W_qkv, W_out):
    from concourse.bass_utils import run_bass_kernel_spmd

    nc = _get_program()
    maps = _host_inputs(x, W_qkv, W_out)
    res = run_bass_kernel_spmd(nc, maps, core_ids=list(range(N_CORES)))
    out = np.zeros((B, S, E), dtype=np.float32)
    for core in range(N_CORES):
        b = core // HG
        out[b] += np.asarray(res.results[core]["out"], dtype=np.float32)
    return out
